# revision 73
# baseline (speedup 1.0000x reference)
"""CARAFE content-aware upsampling on 8 Trainium2 NeuronCores.

Strategy (data parallel, hint-compliant):
  8 cores = 4 batch images x 2 row-halves (32 low-res rows each, +2-row halo).
  Per core, fully fused pipeline in SBUF:
    A) y_down = conv1x1(x, w_down)+b_down        (PE, K=256 in 2 chunks)
    Z) zT = (w_out . x) transposed               (PE produces [col, ch] directly)
    B) enc = conv3x3(y_down, w_enc)              (PE, 9 shifted accum matmuls)
    C) mask = softmax over 25 taps (4 groups)    (PE transpose+group-sums via an
       augmented selector matmul, DVE reciprocal + normalize)
    D) out = sum_k zT[window] * mask  + b_out    (PE: per-row banded matmuls;
       banded mask matrix built by a DRAM-roundtrip diagonal scatter DMA)
  The final 1x1 conv (w_out) is folded BEFORE reassembly (z-trick): conv and
  reassembly commute since both are linear; this runs the big conv at low res
  and skips materializing the upsampled intermediate.

  Scheduling/queue discipline (the perf-critical part):
  - Z's PE-bound matmuls are interleaved into the B/C loop, which is
    otherwise paced by vector/scalar post-processing; the tensor engine
    stays ~100% busy through that phase.
  - The banded-mask DRAM staging uses NSLOT reusable slots zeroed by
    parallel SBUF->DRAM DMAs up front (a serial DRAM->DRAM doubling
    ladder was ~44us of blocking queue time).
  - Reloads fetch only the band columns each w-half matmul actually
    reads (cols 0:640 for w 0..31, 640:1280 for w 32..63), halving
    reload traffic.
  - dma_start costs its issuing engine ~0.7us, so queues are dedicated:
    the scatter->reload chain lives on the sync engine/queue (FIFO
    ordering, engine otherwise idle), early output rows go on the
    latency-tolerant gpsimd software queue, late rows on sync so the
    drain tail is short. Deep obs/psD buffering (opool bufs=20,
    psD bufs=4) keeps the drain engines off the critical path.

Layouts:
  xs     [256, 36, 68]  zero-padded shard (rows h0-2..h1+2, cols -2..65)
  zT     [68, 36, 256]  col-on-partition transpose of z = w_out . x
  B_h    [68, 1280]     banded masks: B[w+j, w*20 + i*4 + p] = mask[h,w,i,j,p]
  out    [256, 64, 128] hi-res shard (fp16 on device, fp32 on host)
"""

import sys
import functools
import numpy as np
from contextlib import ExitStack

for _p in ("/opt/trn_rl_repo",):
    if _p not in sys.path:
        sys.path.insert(0, _p)

import concourse.bass as bass
import concourse.bacc as bacc
import concourse.mybir as mybir
import concourse.tile as tile
from concourse.bass_utils import run_bass_kernel_spmd

NCORES = 8
FP = mybir.dt.float32
BF = mybir.dt.bfloat16
F16 = mybir.dt.float16
AF = mybir.ActivationFunctionType
ALU = mybir.AluOpType

def _ap(base, offset_delta, dims):
    return bass.AP(tensor=base.tensor, offset=base.offset + offset_delta, ap=dims)


@functools.lru_cache(maxsize=1)
def _build():
    nc = bacc.Bacc("TRN2", target_bir_lowering=False, debug=False, num_devices=NCORES)

    xs_d = nc.declare_dram_parameter("xs", [256, 36, 68], BF, isOutput=False)
    wdt_d = nc.declare_dram_parameter("wdt", [256, 128], BF, isOutput=False)
    wet_d = nc.declare_dram_parameter("wet", [128, 9, 100], BF, isOutput=False)
    wot_d = nc.declare_dram_parameter("wot", [256, 256], BF, isOutput=False)
    bd_d = nc.declare_dram_parameter("bd", [128, 1], FP, isOutput=False)
    be_d = nc.declare_dram_parameter("be", [100, 1], FP, isOutput=False)
    bo_d = nc.declare_dram_parameter("bo", [256, 1], FP, isOutput=False)
    saug_d = nc.declare_dram_parameter("saug", [100, 104], BF, isOutput=False)
    edge_d = nc.declare_dram_parameter("edge", [1, 2], FP, isOutput=False)
    out_d = nc.declare_dram_parameter("out", [256, 64, 128], F16, isOutput=True)

    with tile.TileContext(nc) as tc:
        with ExitStack() as ctx:
            const = ctx.enter_context(tc.tile_pool(name="const", bufs=1))
            big = ctx.enter_context(tc.tile_pool(name="big", bufs=1))
            opool = ctx.enter_context(tc.tile_pool(name="opool", bufs=20))
            dpool = ctx.enter_context(tc.tile_pool(name="dpool", bufs=1, space="DRAM"))

            # ---- loads (spread across issuing engines) ----
            # xa/xb split into row chunks so stage A starts on early rows
            # while the rest of the input is still in flight.
            xa = big.tile([128, 36, 68], BF)
            xb = big.tile([128, 36, 68], BF)
            # A-critical constants first on their queues
            wdt = const.tile([128, 2, 128], BF)
            nc.scalar.dma_start(
                out=wdt[:], in_=_ap(wdt_d[:], 0, [[128, 128], [128 * 128, 2], [1, 128]])
            )
            bd = const.tile([128, 1], FP)
            nc.sync.dma_start(out=bd[:], in_=bd_d[:])
            edge = const.tile([128, 2], FP)
            nc.sync.dma_start(
                out=edge[:],
                in_=bass.AP(tensor=edge_d, offset=0, ap=[[0, 128], [1, 2]]),
            )
            engs = (nc.sync, nc.scalar, nc.gpsimd)
            for ci, (r0, r1) in enumerate(((0, 8), (8, 15), (15, 22), (22, 29), (29, 36))):
                engs[(2 * ci) % 3].dma_start(out=xa[:, r0:r1], in_=xs_d[0:128, r0:r1])
                engs[(2 * ci + 1) % 3].dma_start(
                    out=xb[:, r0:r1], in_=xs_d[128:256, r0:r1]
                )
            wet = const.tile([128, 9, 100], BF)
            nc.gpsimd.dma_start(out=wet[:], in_=wet_d[:])
            wot = const.tile([128, 2, 256], BF)
            nc.gpsimd.dma_start(
                out=wot[:], in_=_ap(wot_d[:], 0, [[256, 128], [128 * 256, 2], [1, 256]])
            )
            be = const.tile([100, 1], FP)
            nc.gpsimd.dma_start(out=be[:], in_=be_d[:])
            bo = const.tile([128, 2], FP)
            nc.gpsimd.dma_start(
                out=bo[:], in_=_ap(bo_d[:], 0, [[1, 128], [128, 2]])
            )
            saug = const.tile([100, 104], BF)
            nc.gpsimd.dma_start(out=saug[:], in_=saug_d[:])

            ydown = big.tile([128, 34, 66], BF)
            zt = big.tile([68, 36, 256], BF)
            expv = big.tile([100, 32, 64], BF)
            maskv = big.tile([128, 16, 100], BF)
            inv = big.tile([128, 16, 4], FP)

            # DRAM staging for the banded-mask scatter: NSLOT reusable slots
            # zero-filled by NSLOT parallel SBUF->DRAM DMAs (a serial
            # DRAM->DRAM doubling ladder was ~44us of blocking queue time).
            # Slot h%NSLOT is rewritten at row h+NSLOT, long after row h's
            # reload (batch of 4 rows, issued every 2 row-pairs) completed.
            NSLOT = 8
            SLOT = 68 * 1280
            bstage_all = dpool.tile([NSLOT, 68, 1280], BF, name="bstage_all")
            zero_b = big.tile([68, 1280], BF)
            nc.vector.memset(zero_b[:], 0.0)
            for n in range(NSLOT):
                eng = (nc.scalar, nc.gpsimd)[n % 2]
                eng.dma_start(out=bstage_all[n], in_=zero_b[:])

            # all 32 banded-mask rows live in one SBUF tile; reloads fill
            # disjoint slices so the whole scatter pipeline runs ahead of PE.
            # partitions 0..35 hold band rows 0..35 (w-half 0 windows);
            # partitions 64..99 hold band rows 32..67 (w-half 1 windows) so
            # the two half-row matmuls land on disjoint PE row-groups.
            btX = big.tile([128, 32, 1280], BF)
            # zT columns 30..65 re-based at partition 64 (w-half 1 lhsT)
            zt2 = big.tile([128, 36, 256], BF)

            psum_az = ExitStack()
            psAZ = psum_az.enter_context(tc.tile_pool(name="psAZ", bufs=2, space="PSUM"))
            # ---- stage A: y_down [128ch, 34r, 66c] = w_down . x + b_down ----
            # blocks are interleaved into the B/Z/C loop below, emitted just
            # ahead of the stage-B group that consumes their ydown rows.
            row_blocks = [(0, 6), (6, 12), (12, 18), (18, 24), (24, 30), (30, 34)]

            def stage_a(bi):
                r0, r1 = row_blocks[bi]
                nr = r1 - r0
                pa = psAZ.tile([128, 6, 66], FP, tag="AZ")
                nc.tensor.matmul(
                    pa[:, 0:nr, :], wdt[:, 0, :], xa[:, 1 + r0 : 1 + r1, 1:67],
                    start=True, stop=False,
                )
                nc.tensor.matmul(
                    pa[:, 0:nr, :], wdt[:, 1, :], xb[:, 1 + r0 : 1 + r1, 1:67],
                    start=False, stop=True,
                )
                if r0 == 0:
                    nc.vector.tensor_scalar(
                        ydown[:, 0:1, 1:65], pa[:, 0:1, 1:65], bd[:], edge[:, 0:1],
                        op0=ALU.add, op1=ALU.mult,
                    )
                    nc.scalar.add(ydown[:, 1:6, 1:65], pa[:, 1:6, 1:65], add=bd[:])
                elif r1 == 34:
                    nc.vector.tensor_scalar(
                        ydown[:, 33:34, 1:65], pa[:, 3:4, 1:65], bd[:], edge[:, 1:2],
                        op0=ALU.add, op1=ALU.mult,
                    )
                    nc.scalar.add(ydown[:, 30:33, 1:65], pa[:, 0:3, 1:65], add=bd[:])
                else:
                    if bi % 2 == 0:
                        nc.vector.tensor_scalar(
                            ydown[:, r0:r1, 1:65], pa[:, 0:nr, 1:65], bd[:], None,
                            op0=ALU.add,
                        )
                    else:
                        nc.scalar.add(
                            ydown[:, r0:r1, 1:65], pa[:, 0:nr, 1:65], add=bd[:]
                        )

            # zero the w=-1 / w=64 columns (conv zero-padding semantics);
            # stage_a only ever writes cols 1:65, so these stay zero.
            nc.vector.memset(ydown[:, :, 0:1], 0.0)
            nc.vector.memset(ydown[:, :, 65:66], 0.0)

            # ---- stage Z: zT [68col, 36r, 256ch] = (w_out . x)^T ----
            # interleaved into the B/C loop below (BC is vector/scalar-paced,
            # so Z's PE-bound matmuls fill the idle tensor engine there).
            def stage_z(g):
                pz = psAZ.tile([68, 4, 256], FP, tag="AZ")
                for rr in range(4):
                    r = 4 * g + rr
                    nc.tensor.matmul(
                        pz[:, rr, :], xa[:, r, :], wot[:, 0, :], start=True, stop=False
                    )
                    nc.tensor.matmul(
                        pz[:, rr, :], xb[:, r, :], wot[:, 1, :], start=False, stop=True
                    )
                if g % 2 == 0:
                    nc.vector.tensor_copy(zt[:, 4 * g : 4 * g + 4, :], pz[:])
                else:
                    nc.scalar.copy(zt[:, 4 * g : 4 * g + 4, :], pz[:])
                # w-half-1 re-based copy, batched: 2 DMAs instead of 9
                if g in (4, 8):
                    r0 = 0 if g == 4 else 20
                    r1 = 20 if g == 4 else 36
                    nc.gpsimd.dma_start(
                        out=zt2[64:100, r0:r1, :],
                        in_=zt[32:68, r0:r1, :],
                    )

            # ---- stage B: enc -> exp(enc + b_enc) [100, 32, 64] ----
            def stage_b(b4):
                pb = psB.tile([100, 8, 64], FP, tag="B")
                k = 0
                for di in range(3):
                    for dj in range(3):
                        nc.tensor.matmul(
                            pb[:],
                            wet[:, 3 * di + dj, :],
                            ydown[:, di + 8 * b4 : di + 8 * b4 + 8, dj : dj + 64],
                            start=(k == 0), stop=(k == 8),
                        )
                        k += 1
                nc.scalar.activation(
                    expv[:, 8 * b4 : 8 * b4 + 8, :], pb[:], AF.Exp, bias=be[:]
                )

            # psAZ(4) + psC(2) + psB(2) = 8 banks; all close before psD opens
            psC = psum_az.enter_context(tc.tile_pool(name="psC", bufs=2, space="PSUM"))
            psB = psum_az.enter_context(tc.tile_pool(name="psB", bufs=2, space="PSUM"))

            # ---- stage C (softmax+scatter+reload) interleaved with D ----
            expf = expv[:].rearrange("p a b -> p (a b)")

            def do_reload(k2):
                # whalf0 matmuls read only band cols 0:640 (w 0..31) and
                # whalf1 only cols 640:1280 (w 32..63) -> reload halves.
                rb = ((4 * k2) % NSLOT) * SLOT
                srcA = _ap(
                    bstage_all[:], rb, [[1280, 36], [SLOT, 4], [1, 640]]
                )
                nc.sync.dma_start(
                    out=btX[0:36, 4 * k2 : 4 * k2 + 4, 0:640], in_=srcA
                )
                srcB = _ap(
                    bstage_all[:], rb + 32 * 1280 + 640,
                    [[1280, 36], [SLOT, 4], [1, 640]],
                )
                nc.sync.dma_start(
                    out=btX[64:100, 4 * k2 : 4 * k2 + 4, 640:1280], in_=srcB
                )

            def stage_c(kc):
                pc = psC.tile([128, 104], FP, tag="C")
                nc.tensor.matmul(
                    pc[:],
                    expf[:, 128 * kc : 128 * (kc + 1)],
                    saug[:],
                    start=True, stop=True,
                )
                nc.vector.reciprocal(inv[:, kc, :], pc[:, 100:104])
                inv_b = _ap(inv[:], kc * 4, [[64, 128], [0, 25], [1, 4]])
                nc.vector.tensor_tensor(
                    maskv[:, kc, :].rearrange("p (k q) -> p k q", q=4),
                    pc[:, 0:100].rearrange("p (k q) -> p k q", q=4),
                    inv_b,
                    op=ALU.mult,
                )
                # diagonal scatter per row (DMA APs are limited to 3 dims),
                # rotated across all three DMA-capable engines
                base = ((2 * kc) % NSLOT) * SLOT
                for hh in range(2):
                    dstm = _ap(
                        bstage_all[:], base + hh * SLOT,
                        [[1300, 64], [1280, 5], [1, 20]],
                    )
                    nc.sync.dma_start(
                        out=dstm, in_=maskv[hh * 64 : hh * 64 + 64, kc, :]
                    )
                # batched reload: 4 rows x both partition groups every 2 pairs
                if kc % 2 == 1:
                    do_reload(kc // 2)

            # ---- stage D: banded reassembly + b_out (2-row granularity) ----
            obs = [None, None]

            def stage_d(h):
                if h % 2 == 0:
                    obs[0] = opool.tile([128, 4, 64, 2], F16, tag="ob0", name="ob0")
                    obs[1] = opool.tile([128, 4, 64, 2], F16, tag="ob1", name="ob1")
                for half in range(2):
                    pd2 = psD.tile([128, 1024], FP, tag="D")
                    for i in range(5):
                        rhsA = _ap(
                            btX[:], h * 1280 + 4 * i, [[40960, 36], [20, 32], [1, 4]]
                        )
                        nc.tensor.matmul(
                            pd2[:, 0:128].rearrange("p (w q) -> p w q", q=4),
                            zt[0:36, h + i, 128 * half : 128 * half + 128],
                            rhsA,
                            start=(i == 0), stop=(i == 4),
                        )
                        rhsB = _ap(
                            btX[:],
                            64 * 40960 + h * 1280 + 32 * 20 + 4 * i,
                            [[40960, 36], [20, 32], [1, 4]],
                        )
                        nc.tensor.matmul(
                            pd2[:, 512:640].rearrange("p (w q) -> p w q", q=4),
                            zt2[64:100, h + i, 128 * half : 128 * half + 128],
                            rhsB,
                            start=(i == 0), stop=(i == 4),
                        )
                    ob = obs[half]
                    q = h % 2
                    for whalf in range(2):
                        pd_v = _ap(
                            pd2[:], 512 * whalf, [[1024, 128], [2, 2], [4, 32], [1, 2]]
                        )
                        dst = ob[:, 2 * q : 2 * q + 2, 32 * whalf : 32 * whalf + 32, :]
                        if half == 0:
                            nc.vector.tensor_scalar(
                                dst, pd_v, bo[:, 0:1], None, op0=ALU.add
                            )
                        else:
                            nc.scalar.add(dst, pd_v, add=bo[:, 1:2])
                if h % 2 == 1:
                    for half in range(2):
                        # early rows ride the latency-tolerant gpsimd software
                        # queue; late rows split across the scalar/sync hw
                        # queues so the sync queue (scatter+reload stream)
                        # drains early enough to feed D's last reload batches
                        if h < 16:
                            oeng = nc.gpsimd
                        else:
                            oeng = (nc.scalar, nc.sync)[half]
                        oeng.dma_start(
                            out=out_d[
                                128 * half : 128 * (half + 1),
                                2 * h - 2 : 2 * h + 2,
                                :,
                            ],
                            in_=obs[half][:].rearrange("p a w q -> p a (w q)"),
                        )

            # B/Z/C interleaved: BC is paced by vector/scalar post-processing
            # and the scatter/reload DMA stream, so Z's PE-bound matmuls fill
            # the idle tensor engine; stage D then runs as one matmul stream.
            zmap = {0: (0, 1), 1: (2, 3), 2: (4, 5), 3: (6, 7, 8)}
            for bi in range(6):
                stage_a(bi)
            for b4 in range(4):
                stage_b(b4)
                for g in zmap[b4]:
                    stage_z(g)
                for kc in range(4 * b4, 4 * b4 + 4):
                    stage_c(kc)
            psum_az.close()
            psD = ctx.enter_context(tc.tile_pool(name="psD", bufs=4, space="PSUM"))
            for h in range(32):
                stage_d(h)

    nc.compile()
    return nc


def _host_prep(x, w_down, b_down, w_enc, b_enc, w_out, b_out):
    import ml_dtypes

    bft = ml_dtypes.bfloat16
    x = np.asarray(x, np.float32)
    xp = np.pad(x, [(0, 0), (0, 0), (2, 2), (2, 2)]).astype(bft)
    wdt = np.ascontiguousarray(np.asarray(w_down, np.float32)[:, :, 0, 0].T.astype(bft))
    wet = np.ascontiguousarray(
        np.asarray(w_enc, np.float32).transpose(1, 2, 3, 0).reshape(128, 9, 100)
    ).astype(bft)
    wot = np.ascontiguousarray(np.asarray(w_out, np.float32)[:, :, 0, 0].T.astype(bft))
    bd = np.asarray(b_down, np.float32).reshape(128, 1)
    be = np.asarray(b_enc, np.float32).reshape(100, 1)
    bo = np.asarray(b_out, np.float32).reshape(256, 1)
    # saug: permuted identity (e=(i5,j5,p4) -> e'=(j5,i5,p4)) + 4 group-sum cols
    saug = np.zeros((100, 104), bft)
    for i in range(5):
        for j in range(5):
            for p in range(4):
                saug[(i * 5 + j) * 4 + p, j * 20 + i * 4 + p] = 1.0
    for e in range(100):
        saug[e, 100 + e % 4] = 1.0
    in_maps = []
    for c in range(NCORES):
        n, hh = c // 2, c % 2
        xs = np.ascontiguousarray(xp[n, :, hh * 32 : hh * 32 + 36, :])
        edge = np.array(
            [[0.0 if hh == 0 else 1.0, 0.0 if hh == 1 else 1.0]], np.float32
        )
        in_maps.append(
            dict(xs=xs, wdt=wdt, wet=wet, wot=wot, bd=bd, be=be, bo=bo,
                 saug=saug, edge=edge)
        )
    return in_maps


last_exec_time_ns = None


def kernel(x, w_down, b_down, w_enc, b_enc, w_out, b_out):
    global last_exec_time_ns
    nc = _build()
    in_maps = _host_prep(x, w_down, b_down, w_enc, b_enc, w_out, b_out)
    res = run_bass_kernel_spmd(nc, in_maps, list(range(NCORES)))
    last_exec_time_ns = res.exec_time_ns
    out = np.empty((4, 256, 128, 128), np.float32)
    for c in range(NCORES):
        n, hh = c // 2, c % 2
        out[n, :, hh * 64 : (hh + 1) * 64, :] = np.asarray(
            res.results[c]["out"], np.float32
        )
    return out



# revision 74
# speedup vs baseline: 1.0259x; 1.0259x over previous
"""CARAFE content-aware upsampling on 8 Trainium2 NeuronCores.

Strategy (data parallel, hint-compliant):
  8 cores = 4 batch images x 2 row-halves (32 low-res rows each, +2-row halo).
  Per core, fully fused pipeline in SBUF:
    A) y_down = conv1x1(x, w_down)+b_down        (PE, K=256 in 2 chunks)
    Z) zT = (w_out . x) transposed               (PE produces [col, ch] directly)
    B) enc = conv3x3(y_down, w_enc)              (PE, 9 shifted accum matmuls)
    C) mask = softmax over 25 taps (4 groups)    (PE transpose+group-sums via an
       augmented selector matmul, DVE reciprocal + normalize)
    D) out = sum_k zT[window] * mask  + b_out    (PE: per-row banded matmuls;
       banded mask matrix built by a DRAM-roundtrip diagonal scatter DMA)
  The final 1x1 conv (w_out) is folded BEFORE reassembly (z-trick): conv and
  reassembly commute since both are linear; this runs the big conv at low res
  and skips materializing the upsampled intermediate.

  Scheduling/queue discipline (the perf-critical part):
  - Z's PE-bound matmuls are interleaved into the B/C loop, which is
    otherwise paced by vector/scalar post-processing; the tensor engine
    stays ~100% busy through that phase.
  - The banded-mask DRAM staging uses NSLOT reusable slots zeroed by
    parallel SBUF->DRAM DMAs up front (a serial DRAM->DRAM doubling
    ladder was ~44us of blocking queue time).
  - Reloads fetch only the band columns each w-half matmul actually
    reads (cols 0:640 for w 0..31, 640:1280 for w 32..63), halving
    reload traffic.
  - dma_start costs its issuing engine ~0.7us, so queues are dedicated:
    the scatter->reload chain lives on the sync engine/queue (FIFO
    ordering, engine otherwise idle), early output rows go on the
    latency-tolerant gpsimd software queue, late rows on sync so the
    drain tail is short. Deep obs/psD buffering (opool bufs=20,
    psD bufs=4) keeps the drain engines off the critical path.

Layouts:
  xs     [256, 36, 68]  zero-padded shard (rows h0-2..h1+2, cols -2..65)
  zT     [68, 36, 256]  col-on-partition transpose of z = w_out . x
  B_h    [68, 1280]     banded masks: B[w+j, w*20 + i*4 + p] = mask[h,w,i,j,p]
  out    [256, 64, 128] hi-res shard (fp16 on device, fp32 on host)
"""

import sys
import functools
import numpy as np
from contextlib import ExitStack

for _p in ("/opt/trn_rl_repo",):
    if _p not in sys.path:
        sys.path.insert(0, _p)

import concourse.bass as bass
import concourse.bacc as bacc
import concourse.mybir as mybir
import concourse.tile as tile
from concourse.bass_utils import run_bass_kernel_spmd

NCORES = 8
FP = mybir.dt.float32
BF = mybir.dt.bfloat16
F16 = mybir.dt.float16
AF = mybir.ActivationFunctionType
ALU = mybir.AluOpType

def _ap(base, offset_delta, dims):
    return bass.AP(tensor=base.tensor, offset=base.offset + offset_delta, ap=dims)


@functools.lru_cache(maxsize=1)
def _build():
    nc = bacc.Bacc("TRN2", target_bir_lowering=False, debug=False, num_devices=NCORES)

    xs_d = nc.declare_dram_parameter("xs", [256, 36, 68], BF, isOutput=False)
    wdt_d = nc.declare_dram_parameter("wdt", [256, 128], BF, isOutput=False)
    wet_d = nc.declare_dram_parameter("wet", [128, 9, 100], BF, isOutput=False)
    wot_d = nc.declare_dram_parameter("wot", [256, 256], BF, isOutput=False)
    bd_d = nc.declare_dram_parameter("bd", [128, 1], FP, isOutput=False)
    be_d = nc.declare_dram_parameter("be", [100, 1], FP, isOutput=False)
    bo_d = nc.declare_dram_parameter("bo", [256, 1], FP, isOutput=False)
    saug_d = nc.declare_dram_parameter("saug", [100, 104], BF, isOutput=False)
    edge_d = nc.declare_dram_parameter("edge", [1, 2], FP, isOutput=False)
    out_d = nc.declare_dram_parameter("out", [256, 64, 128], F16, isOutput=True)

    with tile.TileContext(nc) as tc:
        with ExitStack() as ctx:
            const = ctx.enter_context(tc.tile_pool(name="const", bufs=1))
            big = ctx.enter_context(tc.tile_pool(name="big", bufs=1))
            opool = ctx.enter_context(tc.tile_pool(name="opool", bufs=20))
            dpool = ctx.enter_context(tc.tile_pool(name="dpool", bufs=1, space="DRAM"))

            # ---- loads (spread across issuing engines) ----
            # xa/xb split into row chunks so stage A starts on early rows
            # while the rest of the input is still in flight.
            xa = big.tile([128, 36, 68], BF)
            xb = big.tile([128, 36, 68], BF)
            # A-critical constants first on their queues
            wdt = const.tile([128, 2, 128], BF)
            nc.scalar.dma_start(
                out=wdt[:], in_=_ap(wdt_d[:], 0, [[128, 128], [128 * 128, 2], [1, 128]])
            )
            bd = const.tile([128, 1], FP)
            nc.sync.dma_start(out=bd[:], in_=bd_d[:])
            edge = const.tile([128, 2], FP)
            nc.sync.dma_start(
                out=edge[:],
                in_=bass.AP(tensor=edge_d, offset=0, ap=[[0, 128], [1, 2]]),
            )
            engs = (nc.sync, nc.scalar, nc.gpsimd)
            for ci, (r0, r1) in enumerate(((0, 8), (8, 15), (15, 22), (22, 29), (29, 36))):
                engs[(2 * ci) % 3].dma_start(out=xa[:, r0:r1], in_=xs_d[0:128, r0:r1])
                engs[(2 * ci + 1) % 3].dma_start(
                    out=xb[:, r0:r1], in_=xs_d[128:256, r0:r1]
                )
            wet = const.tile([128, 9, 100], BF)
            nc.gpsimd.dma_start(out=wet[:], in_=wet_d[:])
            wot = const.tile([128, 2, 256], BF)
            nc.gpsimd.dma_start(
                out=wot[:], in_=_ap(wot_d[:], 0, [[256, 128], [128 * 256, 2], [1, 256]])
            )
            be = const.tile([100, 1], FP)
            nc.gpsimd.dma_start(out=be[:], in_=be_d[:])
            bo = const.tile([128, 2], FP)
            nc.gpsimd.dma_start(
                out=bo[:], in_=_ap(bo_d[:], 0, [[1, 128], [128, 2]])
            )
            saug = const.tile([100, 104], BF)
            nc.gpsimd.dma_start(out=saug[:], in_=saug_d[:])

            ydown = big.tile([128, 34, 66], BF)
            zt = big.tile([68, 36, 256], BF)
            expv = big.tile([100, 32, 64], BF)
            maskv = big.tile([128, 16, 100], BF)
            inv = big.tile([128, 16, 4], FP)

            # DRAM staging for the banded-mask scatter: NSLOT reusable slots
            # zero-filled by NSLOT parallel SBUF->DRAM DMAs (a serial
            # DRAM->DRAM doubling ladder was ~44us of blocking queue time).
            # Slot h%NSLOT is rewritten at row h+NSLOT, long after row h's
            # reload (batch of 4 rows, issued every 2 row-pairs) completed.
            NSLOT = 8
            SLOT = 68 * 1280
            bstage_all = dpool.tile([NSLOT, 68, 1280], BF, name="bstage_all")
            zero_b = big.tile([68, 1280], BF)
            nc.vector.memset(zero_b[:], 0.0)
            for n in range(NSLOT):
                eng = (nc.scalar, nc.gpsimd)[n % 2]
                eng.dma_start(out=bstage_all[n], in_=zero_b[:])

            # all 32 banded-mask rows live in one SBUF tile; reloads fill
            # disjoint slices so the whole scatter pipeline runs ahead of PE.
            # partitions 0..35 hold band rows 0..35 (w-half 0 windows);
            # partitions 64..99 hold band rows 32..67 (w-half 1 windows) so
            # the two half-row matmuls land on disjoint PE row-groups.
            btX = big.tile([128, 32, 1280], BF)
            # zT columns 30..65 re-based at partition 64 (w-half 1 lhsT)
            zt2 = big.tile([128, 36, 256], BF)

            psum_az = ExitStack()
            psAZ = psum_az.enter_context(tc.tile_pool(name="psAZ", bufs=2, space="PSUM"))
            # ---- stage A: y_down [128ch, 34r, 66c] = w_down . x + b_down ----
            # blocks are interleaved into the B/Z/C loop below, emitted just
            # ahead of the stage-B group that consumes their ydown rows.
            row_blocks = [(0, 6), (6, 12), (12, 18), (18, 24), (24, 30), (30, 34)]

            def stage_a(bi):
                r0, r1 = row_blocks[bi]
                nr = r1 - r0
                pa = psAZ.tile([128, 6, 66], FP, tag="AZ")
                nc.tensor.matmul(
                    pa[:, 0:nr, :], wdt[:, 0, :], xa[:, 1 + r0 : 1 + r1, 1:67],
                    start=True, stop=False,
                )
                nc.tensor.matmul(
                    pa[:, 0:nr, :], wdt[:, 1, :], xb[:, 1 + r0 : 1 + r1, 1:67],
                    start=False, stop=True,
                )
                if r0 == 0:
                    nc.vector.tensor_scalar(
                        ydown[:, 0:1, 1:65], pa[:, 0:1, 1:65], bd[:], edge[:, 0:1],
                        op0=ALU.add, op1=ALU.mult,
                    )
                    nc.scalar.add(ydown[:, 1:6, 1:65], pa[:, 1:6, 1:65], add=bd[:])
                elif r1 == 34:
                    nc.vector.tensor_scalar(
                        ydown[:, 33:34, 1:65], pa[:, 3:4, 1:65], bd[:], edge[:, 1:2],
                        op0=ALU.add, op1=ALU.mult,
                    )
                    nc.scalar.add(ydown[:, 30:33, 1:65], pa[:, 0:3, 1:65], add=bd[:])
                else:
                    if bi % 2 == 0:
                        nc.vector.tensor_scalar(
                            ydown[:, r0:r1, 1:65], pa[:, 0:nr, 1:65], bd[:], None,
                            op0=ALU.add,
                        )
                    else:
                        nc.scalar.add(
                            ydown[:, r0:r1, 1:65], pa[:, 0:nr, 1:65], add=bd[:]
                        )

            # zero the w=-1 / w=64 columns (conv zero-padding semantics);
            # stage_a only ever writes cols 1:65, so these stay zero.
            nc.vector.memset(ydown[:, :, 0:1], 0.0)
            nc.vector.memset(ydown[:, :, 65:66], 0.0)

            # ---- stage Z: zT [68col, 36r, 256ch] = (w_out . x)^T ----
            # interleaved into the B/C loop below (BC is vector/scalar-paced,
            # so Z's PE-bound matmuls fill the idle tensor engine there).
            def stage_z(g):
                pz = psAZ.tile([68, 4, 256], FP, tag="AZ")
                for rr in range(4):
                    r = 4 * g + rr
                    nc.tensor.matmul(
                        pz[:, rr, :], xa[:, r, :], wot[:, 0, :], start=True, stop=False
                    )
                    nc.tensor.matmul(
                        pz[:, rr, :], xb[:, r, :], wot[:, 1, :], start=False, stop=True
                    )
                if g % 2 == 0:
                    nc.vector.tensor_copy(zt[:, 4 * g : 4 * g + 4, :], pz[:])
                else:
                    nc.scalar.copy(zt[:, 4 * g : 4 * g + 4, :], pz[:])
                # w-half-1 re-based copy, batched: 2 DMAs instead of 9
                if g in (4, 8):
                    r0 = 0 if g == 4 else 20
                    r1 = 20 if g == 4 else 36
                    nc.gpsimd.dma_start(
                        out=zt2[64:100, r0:r1, :],
                        in_=zt[32:68, r0:r1, :],
                    )

            # ---- stage B: enc -> exp(enc + b_enc) [100, 32, 64] ----
            def stage_b(b4):
                pb = psB.tile([100, 8, 64], FP, tag="B")
                k = 0
                for di in range(3):
                    for dj in range(3):
                        nc.tensor.matmul(
                            pb[:],
                            wet[:, 3 * di + dj, :],
                            ydown[:, di + 8 * b4 : di + 8 * b4 + 8, dj : dj + 64],
                            start=(k == 0), stop=(k == 8),
                        )
                        k += 1
                nc.scalar.activation(
                    expv[:, 8 * b4 : 8 * b4 + 8, :], pb[:], AF.Exp, bias=be[:]
                )

            # psAZ(4) + psC(2) + psB(2) = 8 banks; all close before psD opens
            psC = psum_az.enter_context(tc.tile_pool(name="psC", bufs=2, space="PSUM"))
            psB = psum_az.enter_context(tc.tile_pool(name="psB", bufs=2, space="PSUM"))

            # ---- stage C (softmax+scatter+reload) interleaved with D ----
            expf = expv[:].rearrange("p a b -> p (a b)")

            def do_reload(k2):
                # whalf0 matmuls read only band cols 0:640 (w 0..31) and
                # whalf1 only cols 640:1280 (w 32..63) -> reload halves.
                rb = ((4 * k2) % NSLOT) * SLOT
                srcA = _ap(
                    bstage_all[:], rb, [[1280, 36], [SLOT, 4], [1, 640]]
                )
                nc.sync.dma_start(
                    out=btX[0:36, 4 * k2 : 4 * k2 + 4, 0:640], in_=srcA
                )
                srcB = _ap(
                    bstage_all[:], rb + 32 * 1280 + 640,
                    [[1280, 36], [SLOT, 4], [1, 640]],
                )
                nc.sync.dma_start(
                    out=btX[64:100, 4 * k2 : 4 * k2 + 4, 640:1280], in_=srcB
                )

            def stage_c(kc):
                pc = psC.tile([128, 104], FP, tag="C")
                nc.tensor.matmul(
                    pc[:],
                    expf[:, 128 * kc : 128 * (kc + 1)],
                    saug[:],
                    start=True, stop=True,
                )
                nc.vector.reciprocal(inv[:, kc, :], pc[:, 100:104])
                inv_b = _ap(inv[:], kc * 4, [[64, 128], [0, 25], [1, 4]])
                nc.vector.tensor_tensor(
                    maskv[:, kc, :].rearrange("p (k q) -> p k q", q=4),
                    pc[:, 0:100].rearrange("p (k q) -> p k q", q=4),
                    inv_b,
                    op=ALU.mult,
                )
                # diagonal scatter per row (DMA APs are limited to 3 dims),
                # rotated across all three DMA-capable engines
                base = ((2 * kc) % NSLOT) * SLOT
                for hh in range(2):
                    dstm = _ap(
                        bstage_all[:], base + hh * SLOT,
                        [[1300, 64], [1280, 5], [1, 20]],
                    )
                    # early scatters ride the (then-empty) sync queue; late
                    # ones move to scalar/gpsimd so the sync queue's reload
                    # stream drains before stage D needs its last batches
                    if kc < 8:
                        seng = nc.sync
                    else:
                        seng = (nc.scalar, nc.gpsimd)[hh]
                    seng.dma_start(
                        out=dstm, in_=maskv[hh * 64 : hh * 64 + 64, kc, :]
                    )
                # batched reload: 4 rows x both partition groups every 2 pairs
                if kc % 2 == 1:
                    do_reload(kc // 2)

            # ---- stage D: banded reassembly + b_out (2-row granularity) ----
            obs = [None, None]

            def stage_d(h):
                if h % 2 == 0:
                    obs[0] = opool.tile([128, 4, 64, 2], F16, tag="ob0", name="ob0")
                    obs[1] = opool.tile([128, 4, 64, 2], F16, tag="ob1", name="ob1")
                for half in range(2):
                    pd2 = psD.tile([128, 1024], FP, tag="D")
                    for i in range(5):
                        rhsA = _ap(
                            btX[:], h * 1280 + 4 * i, [[40960, 36], [20, 32], [1, 4]]
                        )
                        nc.tensor.matmul(
                            pd2[:, 0:128].rearrange("p (w q) -> p w q", q=4),
                            zt[0:36, h + i, 128 * half : 128 * half + 128],
                            rhsA,
                            start=(i == 0), stop=(i == 4),
                        )
                        rhsB = _ap(
                            btX[:],
                            64 * 40960 + h * 1280 + 32 * 20 + 4 * i,
                            [[40960, 36], [20, 32], [1, 4]],
                        )
                        nc.tensor.matmul(
                            pd2[:, 512:640].rearrange("p (w q) -> p w q", q=4),
                            zt2[64:100, h + i, 128 * half : 128 * half + 128],
                            rhsB,
                            start=(i == 0), stop=(i == 4),
                        )
                    ob = obs[half]
                    q = h % 2
                    for whalf in range(2):
                        pd_v = _ap(
                            pd2[:], 512 * whalf, [[1024, 128], [2, 2], [4, 32], [1, 2]]
                        )
                        dst = ob[:, 2 * q : 2 * q + 2, 32 * whalf : 32 * whalf + 32, :]
                        if half == 0:
                            nc.vector.tensor_scalar(
                                dst, pd_v, bo[:, 0:1], None, op0=ALU.add
                            )
                        else:
                            nc.scalar.add(dst, pd_v, add=bo[:, 1:2])
                if h % 2 == 1:
                    for half in range(2):
                        # early rows ride the latency-tolerant gpsimd software
                        # queue; late rows split across the scalar/sync hw
                        # queues so the sync queue (scatter+reload stream)
                        # drains early enough to feed D's last reload batches
                        if h < 16:
                            oeng = nc.gpsimd
                        else:
                            oeng = (nc.scalar, nc.sync)[half]
                        oeng.dma_start(
                            out=out_d[
                                128 * half : 128 * (half + 1),
                                2 * h - 2 : 2 * h + 2,
                                :,
                            ],
                            in_=obs[half][:].rearrange("p a w q -> p a (w q)"),
                        )

            # B/Z/C interleaved: BC is paced by vector/scalar post-processing
            # and the scatter/reload DMA stream, so Z's PE-bound matmuls fill
            # the idle tensor engine; stage D then runs as one matmul stream.
            zmap = {0: (0, 1), 1: (2, 3), 2: (4, 5), 3: (6, 7, 8)}
            for bi in range(6):
                stage_a(bi)
            for b4 in range(4):
                stage_b(b4)
                for g in zmap[b4]:
                    stage_z(g)
                for kc in range(4 * b4, 4 * b4 + 4):
                    stage_c(kc)
            psum_az.close()
            psD = ctx.enter_context(tc.tile_pool(name="psD", bufs=4, space="PSUM"))
            for h in range(32):
                stage_d(h)

    nc.compile()
    return nc


def _host_prep(x, w_down, b_down, w_enc, b_enc, w_out, b_out):
    import ml_dtypes

    bft = ml_dtypes.bfloat16
    x = np.asarray(x, np.float32)
    xp = np.pad(x, [(0, 0), (0, 0), (2, 2), (2, 2)]).astype(bft)
    wdt = np.ascontiguousarray(np.asarray(w_down, np.float32)[:, :, 0, 0].T.astype(bft))
    wet = np.ascontiguousarray(
        np.asarray(w_enc, np.float32).transpose(1, 2, 3, 0).reshape(128, 9, 100)
    ).astype(bft)
    wot = np.ascontiguousarray(np.asarray(w_out, np.float32)[:, :, 0, 0].T.astype(bft))
    bd = np.asarray(b_down, np.float32).reshape(128, 1)
    be = np.asarray(b_enc, np.float32).reshape(100, 1)
    bo = np.asarray(b_out, np.float32).reshape(256, 1)
    # saug: permuted identity (e=(i5,j5,p4) -> e'=(j5,i5,p4)) + 4 group-sum cols
    saug = np.zeros((100, 104), bft)
    for i in range(5):
        for j in range(5):
            for p in range(4):
                saug[(i * 5 + j) * 4 + p, j * 20 + i * 4 + p] = 1.0
    for e in range(100):
        saug[e, 100 + e % 4] = 1.0
    in_maps = []
    for c in range(NCORES):
        n, hh = c // 2, c % 2
        xs = np.ascontiguousarray(xp[n, :, hh * 32 : hh * 32 + 36, :])
        edge = np.array(
            [[0.0 if hh == 0 else 1.0, 0.0 if hh == 1 else 1.0]], np.float32
        )
        in_maps.append(
            dict(xs=xs, wdt=wdt, wet=wet, wot=wot, bd=bd, be=be, bo=bo,
                 saug=saug, edge=edge)
        )
    return in_maps


last_exec_time_ns = None


def kernel(x, w_down, b_down, w_enc, b_enc, w_out, b_out):
    global last_exec_time_ns
    nc = _build()
    in_maps = _host_prep(x, w_down, b_down, w_enc, b_enc, w_out, b_out)
    res = run_bass_kernel_spmd(nc, in_maps, list(range(NCORES)))
    last_exec_time_ns = res.exec_time_ns
    out = np.empty((4, 256, 128, 128), np.float32)
    for c in range(NCORES):
        n, hh = c // 2, c % 2
        out[n, :, hh * 64 : (hh + 1) * 64, :] = np.asarray(
            res.results[c]["out"], np.float32
        )
    return out

